# revision 86
# baseline (speedup 1.0000x reference)
"""Multi-head causal attention (B=4,T=2048,E=1024,H=16,D=64) on 8 TRN2 cores.

Sharding: core c -> batch b=c//2, heads h0=(c%2)*8 .. h0+8. Each core computes
its 8 heads' attention and a partial output projection (row-split Wp); host
sums the two partials per batch (+bias).

Per-core kernel (bf16 matmuls, fp32 PSUM):
  qT/kT = W.T @ x.T              [128, 512] per (pair, j)
  v     = x @ Wv                 [t 128, hd 512] + interleaved ones col
  sT    = kT.T @ qT              [tk 128, tq<=512], 2 chunks per 2-bank psum
  expT  = exp(sT/8)  (one ACT op per chunk pair), causal mask mult on diag
  o     = expT.T @ [v|1]         TRANSPOSED attnV: out [tq 128, 65] (N=65!)
  o/Z   : per-partition recip + scalar-mult (Z = col 64)
  oT    = PE transpose of normalized o pair -> [hd 128, tq]
  part  = oT.T @ WpT             [128, 1024] per t chunk, bf16 partial out

The main loop is a 2-stage software pipeline (scores/exp of head i+1
interleaved with attnV of head i); QKV/output-projection matmul groups are
threaded through it as fine-grained filler sized to the ACT(exp) slack of
each tq tile, so the tensor engine stays dense while ACT runs saturated.
"""
import sys
import numpy as np

sys.path.insert(0, "/opt/trn_rl_repo")

import ml_dtypes
import concourse.bass as bass
import concourse.bacc as bacc
import concourse.mybir as mybir
from concourse import tile
from concourse.bass_utils import run_bass_kernel_spmd

B, T, E, H, D = 4, 2048, 1024, 16, 64
HL = H // 2      # 8 local heads per core
NP = HL // 2     # 4 head pairs
NJ = T // 512    # 4 tq tiles
NCK = T // 128   # 16 t chunks
NE = E // 128    # 8 e chunks
BF16 = mybir.dt.bfloat16
F32 = mybir.dt.float32
bfnp = ml_dtypes.bfloat16

_CACHE = {}


def _build():
    nc = bacc.Bacc("TRN2", target_bir_lowering=False)
    xT = nc.declare_dram_parameter("xT", [E, T], BF16, isOutput=False)
    wq = nc.declare_dram_parameter("wq", [E, HL * D], BF16, isOutput=False)
    wk = nc.declare_dram_parameter("wk", [E, HL * D], BF16, isOutput=False)
    wv = nc.declare_dram_parameter("wv", [E, HL * D], BF16, isOutput=False)
    wpT = nc.declare_dram_parameter("wpT", [HL * D, E], BF16, isOutput=False)
    maskb = nc.declare_dram_parameter("maskb", [128, 128], F32, isOutput=False)
    ident = nc.declare_dram_parameter("ident", [128, 128], BF16, isOutput=False)
    out = nc.declare_dram_parameter("out", [T, E], BF16, isOutput=True)

    Exp = mybir.ActivationFunctionType.Exp

    with tile.TileContext(nc) as tc:
        with (
            tc.tile_pool(name="persist", bufs=1) as pp,
            tc.tile_pool(name="expp", bufs=18) as expp,
            tc.tile_pool(name="zrp", bufs=3) as zrp,
            tc.tile_pool(name="onp", bufs=3) as onp,
            tc.tile_pool(name="obp", bufs=5) as obp,
            tc.tile_pool(name="pap", bufs=4) as pap,
            tc.tile_pool(name="osbp", bufs=3) as osbp,
            tc.tile_pool(name="big", bufs=3, space=bass.MemorySpace.PSUM) as ps_big,
            tc.tile_pool(name="opool", bufs=1, space=bass.MemorySpace.PSUM) as ps_o,
            tc.tile_pool(name="oTp", bufs=1, space=bass.MemorySpace.PSUM) as ps_oT,
        ):
            xT_sb = [pp.tile([128, T], BF16, tag=f"xT{c}", name=f"xT{c}") for c in range(NE)]
            wv_sb = [pp.tile([128, HL * D], BF16, tag=f"wv{c}", name=f"wv{c}") for c in range(NE)]
            wq_sb = [pp.tile([128, HL * D], BF16, tag=f"wq{c}", name=f"wq{c}") for c in range(NE)]
            wk_sb = [pp.tile([128, HL * D], BF16, tag=f"wk{c}", name=f"wk{c}") for c in range(NE)]
            wpT_sb = [pp.tile([128, E], BF16, tag=f"wp{p}", name=f"wp{p}") for p in range(NP)]
            mask_sb = pp.tile([128, 128], F32, tag="mkb", name="mkb")
            id_sb = pp.tile([128, 128], BF16, tag="idn", name="idn")
            v_sb = [pp.tile([128, HL * 65], BF16, tag=f"v{i}", name=f"v{i}") for i in range(NCK)]
            qT_sb = [[pp.tile([128, 512], BF16, tag=f"q{p}_{j}", name=f"q{p}_{j}") for j in range(NJ)]
                     for p in range(NP)]
            kT_sb = [[pp.tile([128, 512], BF16, tag=f"k{p}_{j}", name=f"k{p}_{j}") for j in range(NJ)]
                     for p in range(NP)]
            oT_sb = [[pp.tile([128, 512], BF16, tag=f"o{p}_{j}", name=f"o{p}_{j}") for j in range(NJ)]
                     for p in range(NP)]

            # ---- input DMAs (priority order; interleaved so the first
            # v-projection matmuls can start after just a few transfers) ----
            for c in range(NE):
                nc.sync.dma_start(wv_sb[c][:], wv[c * 128:(c + 1) * 128, :])
                nc.sync.dma_start(xT_sb[c][:, 0:512], xT[c * 128:(c + 1) * 128, 0:512])
            for c in range(NE):
                nc.sync.dma_start(wq_sb[c][:], wq[c * 128:(c + 1) * 128, :])
                nc.sync.dma_start(wk_sb[c][:], wk[c * 128:(c + 1) * 128, :])
            nc.sync.dma_start(mask_sb[:], maskb[:])
            nc.sync.dma_start(id_sb[:], ident[:])
            for c in range(NE):
                nc.sync.dma_start(xT_sb[c][:, 512:1024],
                                  xT[c * 128:(c + 1) * 128, 512:1024])
            for c in range(NE):
                nc.sync.dma_start(xT_sb[c][:, 1024:2048],
                                  xT[c * 128:(c + 1) * 128, 1024:2048])
            for p in range(NP):
                nc.sync.dma_start(wpT_sb[p][:], wpT[p * 128:(p + 1) * 128, :])

            # ones columns for the softmax denominator (gpsimd, idle engine)
            for i in range(NCK):
                nc.gpsimd.memset(v_sb[i][:], 1.0)

            # PE p-state warmup: matmuls with no DMA dependency that span the
            # input-DMA lead-in back-to-back, so the ramp to full clock
            # completes before the first real matmul (bufs=1 pool: no
            # rotation side effects)
            warm = pp.tile([128, 260], BF16, tag="warm", name="warm")
            nc.gpsimd.memset(warm[:], 0.0)
            wscr = ps_o.tile([128, 260], F32, tag="o", name="warmps")
            for _ in range(9):
                nc.tensor.matmul(wscr[:], warm[:, 0:128], warm[:],
                                 start=True, stop=True)



            # ---- filler units (QKV projections + output projection) ----
            # Generators yielding every ~4 matmuls (~850ns of PE work) so
            # filler interleaves finely with the ACT-bound attention loop.
            def v_unit(i):
                big = ps_big.tile([128, 1024], F32, tag="big", name="bigv")
                for c in range(NE):
                    nc.tensor.matmul(big[:, 0:512], xT_sb[c][:, i * 128:(i + 1) * 128],
                                     wv_sb[c][:], start=(c == 0), stop=(c == NE - 1))
                    if c == 3:
                        yield
                v3 = v_sb[i][:].rearrange("p (h d) -> p h d", d=65)
                nc.vector.tensor_copy(
                    v3[:, :, 0:64], big[:, 0:512].rearrange("p (h d) -> p h d", d=64))
                yield

            def qk_unit(p, jj):
                big = ps_big.tile([128, 1024], F32, tag="big", name="bigqk")
                for c in range(NE):
                    nc.tensor.matmul(big[:, 0:512], wq_sb[c][:, p * 128:(p + 1) * 128],
                                     xT_sb[c][:, jj * 512:(jj + 1) * 512],
                                     start=(c == 0), stop=(c == NE - 1))
                    if c == 3:
                        yield
                nc.vector.tensor_copy(qT_sb[p][jj][:], big[:, 0:512])
                yield
                for c in range(NE):
                    nc.tensor.matmul(big[:, 512:1024], wk_sb[c][:, p * 128:(p + 1) * 128],
                                     xT_sb[c][:, jj * 512:(jj + 1) * 512],
                                     start=(c == 0), stop=(c == NE - 1))
                    if c == 3:
                        yield
                nc.vector.tensor_copy(kT_sb[p][jj][:], big[:, 512:1024])
                yield

            def proj_unit(jj, t):
                big = ps_big.tile([128, 1024], F32, tag="big", name="bigpr")
                for n in range(2):
                    for p in range(NP):
                        nc.tensor.matmul(
                            big[:, n * 512:(n + 1) * 512],
                            oT_sb[p][jj][:, (t % 4) * 128:(t % 4 + 1) * 128],
                            wpT_sb[p][:, n * 512:(n + 1) * 512],
                            start=(p == 0), stop=(p == NP - 1))
                    if n == 0:
                        yield
                ob = obp.tile([128, E], BF16, tag="ob", name="ob")
                nc.vector.tensor_copy(ob[:], big[:])
                nc.sync.dma_start(out[t * 128:(t + 1) * 128, :], ob[:])
                yield

            # j=3 projection is split into p01/p23 accumulation halves so the
            # first half can run inside j3's ACT-bound window (shorter tail)
            pa_tiles = {}

            def projA_unit(jj, t):
                big = ps_big.tile([128, 1024], F32, tag="big", name="bigpA")
                for n in range(2):
                    for p in range(2):
                        nc.tensor.matmul(
                            big[:, n * 512:(n + 1) * 512],
                            oT_sb[p][jj][:, (t % 4) * 128:(t % 4 + 1) * 128],
                            wpT_sb[p][:, n * 512:(n + 1) * 512],
                            start=(p == 0), stop=(p == 1))
                    if n == 0:
                        yield
                pa = pap.tile([128, E], BF16, tag="pa", name="pa")
                pa_tiles[t] = pa
                nc.vector.tensor_copy(pa[:], big[:])
                yield

            def projB_unit(jj, t):
                # p2+p3 matmuls, then inject the bf16 p0+p1 partial into the
                # same PSUM group via an identity matmul; copy out on the
                # (tail-idle) ACT engine so the DVE isn't the tail chain.
                big = ps_big.tile([128, 1024], F32, tag="big", name="bigpB")
                for n in range(2):
                    for p in range(2, 4):
                        nc.tensor.matmul(
                            big[:, n * 512:(n + 1) * 512],
                            oT_sb[p][jj][:, (t % 4) * 128:(t % 4 + 1) * 128],
                            wpT_sb[p][:, n * 512:(n + 1) * 512],
                            start=(p == 2), stop=False)
                    nc.tensor.matmul(
                        big[:, n * 512:(n + 1) * 512],
                        id_sb[:],
                        pa_tiles[t][:, n * 512:(n + 1) * 512],
                        start=False, stop=True)
                    if n == 0:
                        yield
                ob = obp.tile([128, E], BF16, tag="ob", name="ob")
                nc.scalar.copy(ob[:], big[:])
                nc.sync.dma_start(out[t * 128:(t + 1) * 128, :], ob[:])
                yield

            from collections import deque
            filler = deque()
            for i in range(4, 8):
                filler.append(("v", i, v_unit(i)))
            for p in range(NP):
                filler.append((f"qk1", p, qk_unit(p, 1)))
            for i in range(8, 12):
                filler.append(("v", i, v_unit(i)))
            for p in range(NP):
                filler.append((f"qk2", p, qk_unit(p, 2)))
            for i in range(12, 16):
                filler.append(("v", i, v_unit(i)))
            for p in range(NP):
                filler.append((f"qk3", p, qk_unit(p, 3)))

            def emit_half(n=1):
                """Emit n ~half-units (~4 matmuls each) of filler work."""
                while n > 0 and filler:
                    kind, idx, gen = filler[0]
                    try:
                        next(gen)
                    except StopIteration:
                        filler.popleft()
                        continue
                    n -= 1

            def drain(pred):
                """Run matching units to completion (dependency deadline)."""
                for ent in list(filler):
                    if not pred(ent[0], ent[1]):
                        continue
                    for _ in ent[2]:
                        pass
                    filler.remove(ent)

            # ---- prephase: v chunks 0-3 and q/k for j=0 ----
            for i in range(4):
                for _ in v_unit(i):
                    pass
            for p in range(NP):
                for _ in qk_unit(p, 0):
                    pass

            # ---- main attention loop (2-stage software pipeline) ----
            # Step i emits scores+exp for head i+1 interleaved with the
            # attnV groups of head i (whose exps completed during step i-1),
            # so the PE never waits on ACT within a step. Filler half-units
            # plug the remaining ACT slack.
            quota = {0: 2, 1: 2, 2: 3, 3: 5}
            on_tiles = {}
            heads = [(j, h) for j in range(NJ) for h in range(HL)]

            def scores_stream(j, h):
                """Yield ets list, growing by one chunk-pair per yield."""
                p, r0 = h // 2, (h % 2) * 64
                ets = []
                for q in range(2 * (j + 1)):
                    big = ps_big.tile([128, 1024], F32, tag="big", name="bigsc")
                    w2 = 0
                    for half in range(2):
                        c = 2 * q + half
                        r = max(0, (c - 4 * j) * 128)
                        w = 512 - r
                        if half == 1:
                            w2 = w
                        nc.tensor.matmul(
                            big[:, half * 512:half * 512 + w],
                            kT_sb[p][c // 4][r0:r0 + 64,
                                             (c % 4) * 128:(c % 4 + 1) * 128],
                            qT_sb[p][j][r0:r0 + 64, r:512],
                            start=True, stop=True)
                    et = expp.tile([128, 1024], BF16, tag="et", name="et")
                    nc.scalar.activation(et[:, 0:512 + w2], big[:, 0:512 + w2],
                                         Exp, scale=0.125)
                    # causal mask on the diagonal (triangle) blocks
                    for half in range(2):
                        c = 2 * q + half
                        if c >= 4 * j:
                            col0 = half * 512
                            nc.vector.tensor_mul(et[:, col0:col0 + 128],
                                                 et[:, col0:col0 + 128],
                                                 mask_sb[:])
                    ets.append(et)
                    yield ets

            def attnv_stream(j, h, ets, o_ps):
                """attnV groups: o[tq,65] += et_chunk.T @ [v|1]; one PSUM
                region's accumulation group per yield (groups must be
                consecutive instruction runs within a bank)."""
                for s in range(4):
                    for c in range(4 * j + s + 1):
                        et = ets[c // 2]
                        half = c % 2
                        r = max(0, (c - 4 * j) * 128)
                        colstart = half * 512 + 128 * s - r
                        nc.tensor.matmul(
                            o_ps[:, 65 * s:65 * s + 65],
                            et[:, colstart:colstart + 128],
                            v_sb[c][:, h * 65:(h + 1) * 65],
                            start=(c == 0),
                            stop=(c == 4 * j + s),
                            skip_group_check=True)
                    yield

            def predrain(jn, hn):
                if jn > 0 and hn == 0:
                    drain(lambda k, i, jj=jn: k == "v" and i <= 4 * jj + 3)
                if hn % 2 == 0 and (jn > 0 or hn > 0):
                    drain(lambda k, i, jj=jn, pp=hn // 2: k == f"qk{jj}" and i == pp)

            # prime the pipeline: scores for the first head
            predrain(*heads[0])
            for ets_cur in scores_stream(*heads[0]):
                pass

            for idx in range(len(heads)):
                j, h = heads[idx]
                p, r0 = h // 2, (h % 2) * 64
                nxt = heads[idx + 1] if idx + 1 < len(heads) else None
                budget = quota[nxt[0]] if nxt else 3
                if nxt:
                    predrain(*nxt)
                    sgen = scores_stream(*nxt)
                else:
                    sgen = None
                o_ps = ps_o.tile([128, 260], F32, tag="o", name="ops")
                agen = attnv_stream(j, h, ets_cur, o_ps)
                ets_next = None
                rr = 0
                while sgen is not None or agen is not None:
                    if sgen is not None:
                        try:
                            ets_next = next(sgen)
                        except StopIteration:
                            sgen = None
                    if agen is not None:
                        try:
                            next(agen)
                        except StopIteration:
                            agen = None
                    rr += 1
                    if rr % 2 == 1 and budget > 0:
                        emit_half(1)
                        budget -= 1

                # stage raw o to SBUF in one copy so the PSUM slot frees
                # fast, then normalize from SBUF (cheaper DVE init too).
                osb = osbp.tile([128, 260], F32, tag="osb", name="osb")
                nc.vector.tensor_copy(osb[:], o_ps[:])
                o3 = osb[:].rearrange("p (s x) -> p s x", x=65)
                zr = zrp.tile([128, 4], F32, tag="zr", name="zr")
                nc.vector.reciprocal(zr[:], o3[:, :, 64:65])
                if h % 2 == 0:
                    on = onp.tile([128, 512], BF16, tag="on", name="on")
                    on_tiles[(p, j)] = on
                else:
                    on = on_tiles[(p, j)]
                on3 = on[:].rearrange("p (s x) -> p s x", x=128)
                if nxt is None:
                    # tail cascade: per-subtile mul -> transpose -> copy ->
                    # projB so the final projection pipelines with the last
                    # head's normalize instead of waiting for all of it
                    drain(lambda k, i: k == "projA")
                    oT_ps = ps_oT.tile([128, 512], BF16, tag="oT", name="oTps")
                    for s in range(4):
                        nc.vector.tensor_scalar_mul(
                            on3[:, s:s + 1, r0:r0 + 64], o3[:, s:s + 1, 0:64],
                            zr[:, s:s + 1])
                        nc.tensor.transpose(oT_ps[:, 128 * s:128 * (s + 1)],
                                            on[:, 128 * s:128 * (s + 1)],
                                            id_sb[:])
                        nc.vector.tensor_copy(
                            oT_sb[p][j][:, 128 * s:128 * (s + 1)],
                            oT_ps[:, 128 * s:128 * (s + 1)])
                    for s in range(4):
                        for _ in projB_unit(j, 12 + s):
                            pass
                    continue
                for s in range(4):
                    nc.vector.tensor_scalar_mul(
                        on3[:, s:s + 1, r0:r0 + 64], o3[:, s:s + 1, 0:64],
                        zr[:, s:s + 1])

                # filler before the transposes so the PE doesn't sit in
                # line behind the DVE normalize chain
                emit_half(budget)

                if h % 2 == 1:
                    # pair complete: PE-transpose [tq,hd]->[hd,tq], stash to SBUF
                    oT_ps = ps_oT.tile([128, 512], BF16, tag="oT", name="oTps")
                    for s in range(4):
                        nc.tensor.transpose(oT_ps[:, 128 * s:128 * (s + 1)],
                                            on[:, 128 * s:128 * (s + 1)],
                                            id_sb[:])
                    nc.vector.tensor_copy(oT_sb[p][j][:], oT_ps[:])

                if j == 3 and h == 5:
                    # pairs 0,1 done earlier: first half of j3's projection
                    # (appended late so filler remains for the last steps)
                    for t in range(12, 16):
                        filler.append(("projA", t, projA_unit(j, t)))

                if h == HL - 1 and j < 3:
                    # output projection for this j becomes available as filler
                    for t in range(4 * j, 4 * j + 4):
                        filler.append(("proj", t, proj_unit(j, t)))

                ets_cur = ets_next

            while filler:
                emit_half(4)

    nc.compile()
    return nc


def _masks_np():
    f = np.arange(128)[None, :]
    p = np.arange(128)[:, None]
    return (f >= p).astype(np.float32)


def kernel(x, Wq, Wk, Wv, Wp, bp):
    x = np.asarray(x, dtype=np.float32)
    Wq = np.asarray(Wq, dtype=np.float32)
    Wk = np.asarray(Wk, dtype=np.float32)
    Wv = np.asarray(Wv, dtype=np.float32)
    Wp = np.asarray(Wp, dtype=np.float32)
    bp = np.asarray(bp, dtype=np.float32)

    if "nc" not in _CACHE:
        _CACHE["nc"] = _build()
    nc = _CACHE["nc"]

    masks = _masks_np()
    identity = np.eye(128, dtype=bfnp)
    WpT = np.ascontiguousarray(Wp.T).astype(bfnp)  # [E(hd), E(n)]
    xTs = [np.ascontiguousarray(x[b].T).astype(bfnp) for b in range(B)]

    def wslice(W, h0):  # [H,E,D] -> [E, 8*64] col = local head*64+d
        return np.ascontiguousarray(
            W[h0:h0 + HL].transpose(1, 0, 2).reshape(E, HL * D)).astype(bfnp)

    in_maps = []
    for c in range(8):
        b, hh = c // 2, c % 2
        h0 = hh * HL
        in_maps.append({
            "xT": xTs[b],
            "wq": wslice(Wq, h0),
            "wk": wslice(Wk, h0),
            "wv": wslice(Wv, h0),
            "wpT": np.ascontiguousarray(WpT[h0 * D:(h0 + HL) * D, :]),
            "maskb": masks,
            "ident": identity,
        })

    res = run_bass_kernel_spmd(nc, in_maps, list(range(8)))
    parts = [np.asarray(res.results[c]["out"], dtype=np.float32) for c in range(8)]
    out = np.stack([parts[2 * b] + parts[2 * b + 1] for b in range(B)], axis=0)
    return (out + bp[None, None, :]).astype(np.float32)
